# revision 10
# baseline (speedup 1.0000x reference)
"""Trainium2 Bass kernel: CrossSRA (spatial-reduction cross attention).

Sharding: data-parallel over batch B=8 across the 8 NeuronCores for
q/attention/proj; the spatial-reduction conv contraction (C_in x 8x8
patch) is split by input channel across cores (each core computes a
partial sum of the reduced tokens for ALL batches), combined with a
ReduceScatter that hands each core exactly its own batch's tokens.

All matmuls run in fp32r (full PE rate at free-dim 512).  Matmul
outputs must start at PSUM partition 0 on this toolchain (no column
tile_position), so head-pair packing is done with block-diagonal
stationary operands instead:

  scores  : kbd[hp] = diag(kA, kB) [128d, 128m] -> one K=128 matmul
            yields both heads' scores [mA|mB, n] in one PSUM tile.
  sums    : obd [128, 64] (ones blocks) -> rows 0-31 = sum over mA,
            rows 32-63 = sum over mB, replicated.
  inject  : nbd [64, 128] with -1 at (0, 0:64) and (32, 64:128):
            one K=64 matmul adds -ln(sum_h[n]) to the right head's
            score rows, so pass-2 exp() directly emits normalized
            attention weights.
  attn@v  : vbd[hp] = diag(vA, vB) [128m, 128d] -> one matmul lands
            the output in the transposed [c, n] layout proj consumes.

proj then emits y in natural [n, c] layout, written straight to DRAM.
"""

import sys

sys.path.insert(0, "/opt/trn_rl_repo")

from contextlib import ExitStack

import numpy as np

import concourse.bass as bass
import concourse.tile as tile
from concourse import bacc, mybir
from concourse.bass_utils import run_bass_kernel_spmd
from concourse.masks import make_identity

# Problem constants (hardcoded per spec nn_CrossSRA_42202348650882)
B, N, C = 8, 4096, 512
NH, D = 8, 64          # heads, head dim
SR = 8                 # spatial reduction ratio
M = 64                 # reduced token count (64/8 * 64/8)
NCORES = 8
CSL = C // NCORES      # conv input-channel slice per core
TN = 512               # n-tile size
NT = N // TN
SCALE = float(D) ** -0.5

F32 = mybir.dt.float32
F32R = mybir.dt.float32r
AF = mybir.ActivationFunctionType
ALU = mybir.AluOpType


def _bcast(ap1d, p):
    """Broadcast a 1-D AP across p partitions (stride-0 partition dim)."""
    return bass.AP(tensor=ap1d.tensor, offset=ap1d.offset, ap=[[0, p]] + list(ap1d.ap))


def build_program(sim_mode: bool = False):
    """Build the SPMD Bass program (identical on all 8 cores)."""
    nc = bacc.Bacc(
        "TRN2", target_bir_lowering=False, debug=False, num_devices=NCORES
    )

    # --- DRAM parameters (per-core inputs prepared on host) ---
    xvT = nc.declare_dram_parameter("xvT", [C, N], F32R, isOutput=False)
    xp = nc.declare_dram_parameter("xp", [4096, 512], F32R, isOutput=False)
    wc = nc.declare_dram_parameter("wc", [4096, 512], F32R, isOutput=False)
    qwT = nc.declare_dram_parameter("qwT", [C, C], F32R, isOutput=False)
    kwT = nc.declare_dram_parameter("kwT", [C, C], F32R, isOutput=False)
    vwT = nc.declare_dram_parameter("vwT", [C, C], F32R, isOutput=False)
    pwT = nc.declare_dram_parameter("pwT", [C, C], F32R, isOutput=False)
    obd = nc.declare_dram_parameter("obd", [128, 64], F32R, isOutput=False)
    nbd = nc.declare_dram_parameter("nbd", [64, 128], F32R, isOutput=False)
    qb_s = nc.declare_dram_parameter("qb_s", [C], F32, isOutput=False)
    kb = nc.declare_dram_parameter("kb", [C], F32, isOutput=False)
    vb = nc.declare_dram_parameter("vb", [C], F32, isOutput=False)
    pb = nc.declare_dram_parameter("pb", [C], F32, isOutput=False)
    srb = nc.declare_dram_parameter("srb", [C], F32, isOutput=False)
    lng = nc.declare_dram_parameter("lng", [C], F32, isOutput=False)
    lnb = nc.declare_dram_parameter("lnb", [C], F32, isOutput=False)
    y = nc.declare_dram_parameter("y", [N, C], F32, isOutput=True)

    cc_in = nc.dram_tensor("cc_in", [B * M, C], F32)
    cc_out = nc.dram_tensor("cc_out", [M, C], F32)

    with tile.TileContext(nc) as tc, ExitStack() as ctx:
        # ---------- constants ----------
        consts = ctx.enter_context(tc.tile_pool(name="consts", bufs=1))
        identity = consts.tile([128, 128], F32)
        make_identity(nc, identity)
        obd_t = consts.tile([128, 64], F32R)
        nc.sync.dma_start(out=obd_t, in_=obd[:, :])
        nbd_t = consts.tile([64, 128], F32R)
        nc.sync.dma_start(out=nbd_t, in_=nbd[:, :])

        qb_t = consts.tile([128, 4], F32)
        nc.sync.dma_start(out=qb_t, in_=qb_s.ap().rearrange("(g p) -> p g", p=128))
        kb_t = consts.tile([128, 4], F32)
        nc.sync.dma_start(out=kb_t, in_=kb.ap().rearrange("(g p) -> p g", p=128))

        pb_bc = consts.tile([128, C], F32)
        nc.sync.dma_start(out=pb_bc, in_=_bcast(pb.ap(), 128))
        vb_bc = consts.tile([128, C], F32)
        nc.sync.dma_start(out=vb_bc, in_=_bcast(vb.ap(), 128))
        srb_bc = consts.tile([64, C], F32)
        nc.sync.dma_start(out=srb_bc, in_=_bcast(srb.ap(), 64))
        lng_bc = consts.tile([64, C], F32)
        nc.sync.dma_start(out=lng_bc, in_=_bcast(lng.ap(), 64))
        lnb_bc = consts.tile([64, C], F32)
        nc.sync.dma_start(out=lnb_bc, in_=_bcast(lnb.ap(), 64))

        # ---------- phase A: spatial-reduction conv (c-slice partial) ----------
        with (
            tc.tile_pool(name="conv", bufs=1) as convp,
            tc.tile_pool(name="cps", bufs=2, space="PSUM") as cps,
        ):
            wc_t = convp.tile([128, 32, 512], F32R)
            xp_t = convp.tile([128, 32, 512], F32R)
            wc_r = wc.ap().rearrange("(k p) o -> p k o", p=128)
            xp_r = xp.ap().rearrange("(k p) o -> p k o", p=128)
            for g in range(8):
                ks = slice(4 * g, 4 * g + 4)
                nc.sync.dma_start(out=wc_t[:, ks, :], in_=wc_r[:, ks, :])
                nc.sync.dma_start(out=xp_t[:, ks, :], in_=xp_r[:, ks, :])

            tokp_t = convp.tile([128, 4, 512], F32)
            for pair in range(4):
                ps = cps.tile([128, 512], F32)
                cols = slice(pair * 128, (pair + 1) * 128)
                for k in range(32):
                    nc.tensor.matmul(
                        ps,
                        xp_t[:, k, cols],
                        wc_t[:, k, :],
                        start=(k == 0),
                        stop=(k == 31),
                    )
                nc.vector.tensor_copy(tokp_t[:, pair, :], ps)
                nc.sync.dma_start(
                    out=cc_in[pair * 128 : (pair + 1) * 128, :],
                    in_=tokp_t[:, pair, :],
                )

            if sim_mode:
                nc.sync.dma_start(out=cc_out[:, :], in_=cc_in[0:M, :])
            else:
                nc.gpsimd.collective_compute(
                    "ReduceScatter",
                    ALU.add,
                    replica_groups=[list(range(NCORES))],
                    ins=[cc_in[:, :]],
                    outs=[cc_out[:, :]],
                )

        # ---------- phase B: q projection (transposed layout) ----------
        qpool = ctx.enter_context(tc.tile_pool(name="qT", bufs=1))
        q_t = qpool.tile([128, 4, NT, TN], F32R)  # [p, c_out blk, nt, n]
        with (
            tc.tile_pool(name="qw", bufs=1) as qwp,
            tc.tile_pool(name="xv", bufs=3) as xvp,
            tc.tile_pool(name="qps", bufs=2, space="PSUM") as qps,
        ):
            qw_t = qwp.tile([128, 4, 512], F32R)
            nc.sync.dma_start(
                out=qw_t, in_=qwT.ap().rearrange("(ci p) o -> p ci o", p=128)
            )
            xv_r = xvT.ap().rearrange("(cb p) n -> p cb n", p=128)
            for nt in range(NT):
                xv_t = xvp.tile([128, 4, TN], F32R)
                nc.sync.dma_start(
                    out=xv_t, in_=xv_r[:, :, nt * TN : (nt + 1) * TN]
                )
                for co in range(4):
                    ps = qps.tile([128, TN], F32)
                    for ci in range(4):
                        nc.tensor.matmul(
                            ps,
                            qw_t[:, ci, co * 128 : (co + 1) * 128],
                            xv_t[:, ci, :],
                            start=(ci == 0),
                            stop=(ci == 3),
                        )
                    nc.vector.tensor_scalar_add(
                        q_t[:, co, nt, :], ps, qb_t[:, co : co + 1]
                    )

        # ---------- phase C: tokens -> LN -> kbd, vbd (block-diagonal) ----------
        kvpool = ctx.enter_context(tc.tile_pool(name="kv", bufs=1))
        tokT2_t = kvpool.tile([128, 4, 128], F32R)  # tokens^T, col-duplicated
        kbd_ts = [
            kvpool.tile([128, 128], F32R, name=f"kbd{i}", tag=f"kbd{i}") for i in range(4)
        ]
        vbd_ts = [
            kvpool.tile([128, 128], F32R, name=f"vbd{i}", tag=f"vbd{i}") for i in range(4)
        ]
        pw_t = kvpool.tile([128, 4, 512], F32R)
        nc.sync.dma_start(
            out=pw_t, in_=pwT.ap().rearrange("(ci p) o -> p ci o", p=128)
        )
        with (
            tc.tile_pool(name="tokw", bufs=1) as tokp,
            tc.tile_pool(name="kvps", bufs=4, space="PSUM") as kvps,
        ):
            kw_t = tokp.tile([128, 4, 512], F32R)
            vw_t = tokp.tile([128, 4, 512], F32R)
            nc.sync.dma_start(
                out=kw_t, in_=kwT.ap().rearrange("(ci p) o -> p ci o", p=128)
            )
            nc.sync.dma_start(
                out=vw_t, in_=vwT.ap().rearrange("(ci p) o -> p ci o", p=128)
            )
            tok_t = tokp.tile([M, C], F32)
            nc.sync.dma_start(out=tok_t, in_=cc_out[:, :])
            nc.vector.tensor_add(tok_t, tok_t, srb_bc)

            stats = tokp.tile([M, 6], F32)
            nc.vector.bn_stats(out=stats, in_=tok_t)
            mv = tokp.tile([M, 2], F32)
            nc.vector.bn_aggr(out=mv, in_=stats)
            std = tokp.tile([M, 1], F32)
            nc.vector.tensor_scalar_add(std, mv[:, 1:2], 1e-5)
            nc.scalar.sqrt(std, std)
            rstd = tokp.tile([M, 1], F32)
            nc.vector.reciprocal(rstd, std)
            nc.vector.tensor_scalar(
                out=tok_t,
                in0=tok_t,
                scalar1=mv[:, 0:1],
                scalar2=rstd,
                op0=ALU.subtract,
                op1=ALU.mult,
            )
            nc.vector.tensor_mul(tok_t, tok_t, lng_bc)
            nc.vector.tensor_add(tok_t, tok_t, lnb_bc)

            # transpose tokens -> tokT2 [c, m|m] (column-duplicated)
            for cb in range(4):
                pst = kvps.tile([128, M], F32)
                nc.tensor.transpose(
                    pst, tok_t[:, cb * 128 : (cb + 1) * 128], identity[0:M, 0:M]
                )
                nc.vector.tensor_copy(tokT2_t[:, cb, 0:64], pst)
                nc.vector.tensor_copy(tokT2_t[:, cb, 64:128], pst)

            # kbd[hp]: k^T for head pair, block-diagonalized
            for hp in range(4):
                ps = kvps.tile([128, 128], F32)
                for ci in range(4):
                    nc.tensor.matmul(
                        ps,
                        kw_t[:, ci, hp * 128 : (hp + 1) * 128],
                        tokT2_t[:, ci, :],
                        start=(ci == 0),
                        stop=(ci == 3),
                    )
                kbd = kbd_ts[hp]
                nc.vector.tensor_scalar_add(
                    kbd[0:64, 0:64], ps[0:64, 0:64], kb_t[0:64, hp : hp + 1]
                )
                nc.vector.tensor_scalar_mul(kbd[0:64, 64:128], ps[0:64, 64:128], 0.0)
                nc.vector.tensor_scalar_mul(kbd[64:128, 0:64], ps[64:128, 0:64], 0.0)
                nc.vector.tensor_scalar_add(
                    kbd[64:128, 64:128], ps[64:128, 64:128], kb_t[64:128, hp : hp + 1]
                )

            # vbd[hp]: v for head pair, block-diagonalized
            for hp in range(4):
                ps = kvps.tile([128, 128], F32)
                for ci in range(4):
                    nc.tensor.matmul(
                        ps,
                        tokT2_t[:, ci, :],
                        vw_t[:, ci, hp * 128 : (hp + 1) * 128],
                        start=(ci == 0),
                        stop=(ci == 3),
                    )
                vbd = vbd_ts[hp]
                nc.vector.tensor_add(
                    vbd[0:64, 0:64], ps[0:64, 0:64],
                    vb_bc[0:64, hp * 128 : hp * 128 + 64],
                )
                nc.vector.tensor_scalar_mul(vbd[0:64, 64:128], ps[0:64, 64:128], 0.0)
                nc.vector.tensor_scalar_mul(vbd[64:128, 0:64], ps[64:128, 0:64], 0.0)
                nc.vector.tensor_add(
                    vbd[64:128, 64:128], ps[64:128, 64:128],
                    vb_bc[64:128, hp * 128 + 64 : hp * 128 + 128],
                )

        # ---------- phase D: attention + proj, software-pipelined over nt ----------
        with (
            tc.tile_pool(name="scps", bufs=2, space="PSUM") as scps,
            tc.tile_pool(name="smps", bufs=1, space="PSUM") as smps,
            tc.tile_pool(name="yps", bufs=2, space="PSUM") as yps,
            tc.tile_pool(name="e1p", bufs=5) as e1p,
            tc.tile_pool(name="e2p", bufs=5) as e2p,
            tc.tile_pool(name="Lp", bufs=2) as Lp,
            tc.tile_pool(name="osb", bufs=2) as osb,
            tc.tile_pool(name="ysb", bufs=3) as ysb,
        ):

            def emit_pass1(nt):
                """Scores pass 1 -> exp -> per-head sums -> L = ln(sum)."""
                e1_ts = []
                for hp in range(4):
                    ps = scps.tile([128, TN], F32, name=f"s1_{nt}_{hp}", tag="sc")
                    nc.tensor.matmul(
                        ps, kbd_ts[hp], q_t[:, hp, nt, :], start=True, stop=True
                    )
                    e1 = e1p.tile([128, TN], F32R, name=f"e1_{nt}_{hp}", tag="e1")
                    nc.scalar.activation(e1, ps, AF.Exp)
                    e1_ts.append(e1)
                sums = smps.tile([64, 4, TN], F32, name=f"sums{nt}", tag="sums")
                for hp in range(4):
                    nc.tensor.matmul(
                        sums[:, hp, :],
                        obd_t,
                        e1_ts[hp],
                        start=True,
                        stop=True,
                    )
                L = Lp.tile([64, 4, TN], F32R, name=f"L{nt}", tag="L")
                nc.scalar.activation(L, sums, AF.Ln)
                return L

            def emit_pass2(nt, L):
                """Scores pass 2 with -L injection -> exp -> attn@v -> proj."""
                e2_ts = []
                for hp in range(4):
                    ps = scps.tile([128, TN], F32, name=f"s2_{nt}_{hp}", tag="sc")
                    nc.tensor.matmul(
                        ps, kbd_ts[hp], q_t[:, hp, nt, :], start=True, stop=False
                    )
                    nc.tensor.matmul(
                        ps, nbd_t, L[:, hp, :], start=False, stop=True
                    )
                    e2 = e2p.tile([128, TN], F32R, name=f"e2_{nt}_{hp}", tag="e2")
                    nc.scalar.activation(e2, ps, AF.Exp)
                    e2_ts.append(e2)

                o_t = osb.tile([128, 4, TN], F32R, name=f"o{nt}", tag="o")
                for hp in range(4):
                    ps = scps.tile([128, TN], F32, name=f"ov_{nt}_{hp}", tag="sc")
                    nc.tensor.matmul(
                        ps, vbd_ts[hp], e2_ts[hp], start=True, stop=True
                    )
                    nc.vector.tensor_copy(o_t[:, hp, :], ps)

                for nn in range(4):
                    ps = yps.tile([128, C], F32, name=f"y_{nt}_{nn}", tag="y")
                    for cb in range(4):
                        nc.tensor.matmul(
                            ps,
                            o_t[:, cb, nn * 128 : (nn + 1) * 128],
                            pw_t[:, cb, :],
                            start=(cb == 0),
                            stop=(cb == 3),
                        )
                    yt = ysb.tile([128, C], F32, name=f"yt_{nt}_{nn}", tag="yt")
                    nc.vector.tensor_add(yt, ps, pb_bc)
                    r0 = nt * TN + nn * 128
                    nc.sync.dma_start(out=y[r0 : r0 + 128, :], in_=yt)

            prev = None
            for nt in range(NT):
                L = emit_pass1(nt)
                if prev is not None:
                    emit_pass2(prev[0], prev[1])
                prev = (nt, L)
            emit_pass2(prev[0], prev[1])

    nc.compile()
    return nc


_NC_CACHE = {}


def _get_nc(sim_mode=False):
    key = bool(sim_mode)
    if key not in _NC_CACHE:
        _NC_CACHE[key] = build_program(sim_mode=key)
    return _NC_CACHE[key]


def make_bd_consts():
    """Host-side block-structured constants for sums + inject matmuls."""
    obd_host = np.zeros((128, 64), np.float32)
    obd_host[0:64, 0:32] = 1.0    # col f<32: sum over head-A rows
    obd_host[64:128, 32:64] = 1.0  # col 32<=f<64: sum over head-B rows
    nbd_host = np.zeros((64, 128), np.float32)
    nbd_host[0, 0:64] = -1.0      # head-A score rows get -L_A (L row 0)
    nbd_host[32, 64:128] = -1.0   # head-B score rows get -L_B (L row 32)
    return obd_host, nbd_host


def prep_in_maps(inputs):
    """Host-side sharding/layout prep -> list of per-core input dicts."""
    x_vis = np.ascontiguousarray(np.asarray(inputs["x_vis"], dtype=np.float32))
    x_ir = np.ascontiguousarray(np.asarray(inputs["x_ir"], dtype=np.float32))
    qW = np.asarray(inputs["qW"], dtype=np.float32)
    kW = np.asarray(inputs["kW"], dtype=np.float32)
    vW = np.asarray(inputs["vW"], dtype=np.float32)
    projW = np.asarray(inputs["projW"], dtype=np.float32)
    srW = np.asarray(inputs["srW"], dtype=np.float32)
    qb = np.asarray(inputs["qb"], dtype=np.float32)
    kb_ = np.asarray(inputs["kb"], dtype=np.float32)
    vb_ = np.asarray(inputs["vb"], dtype=np.float32)
    pb_ = np.asarray(inputs["projb"], dtype=np.float32)
    srb_ = np.asarray(inputs["srb"], dtype=np.float32)
    lng_ = np.asarray(inputs["ln_g"], dtype=np.float32)
    lnb_ = np.asarray(inputs["ln_b"], dtype=np.float32)
    assert int(inputs["H"]) == 64 and int(inputs["W"]) == 64
    assert x_vis.shape == (B, N, C)

    qwT = np.ascontiguousarray((qW * SCALE).T)
    kwT = np.ascontiguousarray(kW.T)
    vwT = np.ascontiguousarray(vW.T)
    pwT = np.ascontiguousarray(projW.T)
    qb_s = np.ascontiguousarray(qb * SCALE)
    obd_host, nbd_host = make_bd_consts()

    # conv X side: [kh, kw, c, b, i, j]
    xr = np.ascontiguousarray(
        x_ir.reshape(B, 8, 8, 8, 8, C).transpose(2, 4, 5, 0, 1, 3)
    )
    # conv W side: [p, c, o]
    wr = np.ascontiguousarray(srW.reshape(C, C, 64).transpose(2, 1, 0))

    shared = dict(
        qwT=qwT, kwT=kwT, vwT=vwT, pwT=pwT, qb_s=qb_s, kb=kb_, vb=vb_,
        pb=pb_, srb=srb_, lng=lng_, lnb=lnb_, obd=obd_host, nbd=nbd_host,
    )
    in_maps = []
    for core in range(NCORES):
        cs = slice(core * CSL, (core + 1) * CSL)
        m = dict(shared)
        m["xvT"] = np.ascontiguousarray(x_vis[core].T)
        m["xp"] = np.ascontiguousarray(xr[:, :, cs].reshape(64 * CSL, B * M))
        m["wc"] = np.ascontiguousarray(wr[:, cs, :].reshape(64 * CSL, C))
        in_maps.append(m)
    return in_maps


def kernel(**inputs):
    nc = _get_nc(sim_mode=False)
    in_maps = prep_in_maps(inputs)
    res = run_bass_kernel_spmd(nc, in_maps, list(range(NCORES)))
    out = np.stack([res.results[c]["y"] for c in range(NCORES)], axis=0)
    return out.astype(np.float32)
